# revision 17
# baseline (speedup 1.0000x reference)
"""Trainium2 Bass kernel for nn_ConsumptionPredictor.

Model: 2-layer LSTM (H=5, S=512) over batch 16384, then two linear layers
with no intervening nonlinearity (W1/W2 collapse into a single 2560-dim
dot product v = W2 @ W1, c0 = W2 @ b1 + b2).

Strategy (per core, batch 2048 = 1/8 of 16384):
  * Sequence chunking: C=4 chunks of L=128 steps, each warmed up with W
    extra leading steps (forget-gate decay makes warmup error tiny).  All
    4 chunks x 2 batch halves advance in lockstep as G=8 groups of 1024
    samples -> virtual batch 8192 per step.
  * Both LSTM layers fused in one step loop (layer 1 lags layer 0 by one
    step).  Unified fp16 state tile hs (88, 1024): rows 0:40 h1 (8 groups
    x 5), 40:80 h0, 80:88 x (prefetched one step ahead by DMA).
  * Per-gate PSUM tiles (80, 1024) pack BOTH layers contiguously
    ([l1, l0]) so one matmul per gate per 512-slice computes both layers
    (lhsT packs Whh/Wih blocks, K=88), and every ACT/DVE op is a single
    base-0 80-row instruction (partition bases must be 0/32/64/96).
  * Scalar: 4 gate activations + tanh(c), each free-size 1024, with
    per-partition bias.  DVE: 4 fp16 tensor_tensor ops (2x mode).
  * Readout acc (out[b] += v[s].h1[b,s]) accumulated per step by a small
    matmul into rows 96:104 of the o-gate PSUM tile (fits the 8-bank
    budget); final 4-chunk reduction host-side.
  * Chunk-0 state resets at t=W / t=W+1 via per-partition mask multiplies
    (memset at partition base 40 would violate the alignment rule).
"""

import sys

import numpy as np

try:
    import concourse.bass as bass  # noqa: F401
except ImportError:  # pragma: no cover
    sys.path.insert(0, "/opt/trn_rl_repo")

import concourse.bass as bass
import concourse.tile as tile
from concourse import bacc, mybir
from concourse.bass_utils import run_bass_kernel_spmd

# ----- problem/config constants (hardcoded; kernel.py must be self-contained)
NCORES = 8
B, S, H = 16384, 512, 5
BC = B // NCORES          # 2048 samples per core
C = 4                     # sequence chunks
L = S // C                # 128 steps per chunk
W = 8                     # warmup steps per chunk
T = W + L + 1             # virtual steps (layer 1 lags by one)
G = 8                     # groups (4 chunks x 2 batch halves)
BF = 1024                 # free width per group
J = 2                     # 512-wide column slices (PSUM bank width)
JS = BF // J
F32 = mybir.dt.float32
F16 = mybir.dt.float16
Sig = mybir.ActivationFunctionType.Sigmoid
Tanh = mybir.ActivationFunctionType.Tanh
MUL = mybir.AluOpType.mult
ADD = mybir.AluOpType.add

_CACHE = {}


def _build_program():
    if "nc" in _CACHE:
        return _CACHE["nc"]

    nc = bacc.Bacc("TRN2", target_bir_lowering=False, debug=False,
                   enable_asserts=False, num_devices=NCORES)

    xT = nc.dram_tensor("xT", [W + S + 8, BC], F16, kind="ExternalInput")
    lhsT_d = nc.dram_tensor("lhsT", [88, 320], F16, kind="ExternalInput")
    bias_d = nc.dram_tensor("bias", [80, 4], F32, kind="ExternalInput")
    mask_d = nc.dram_tensor("mask", [80, 2], F32, kind="ExternalInput")
    vtab_d = nc.dram_tensor("vtab", [40, L * G], F16, kind="ExternalInput")
    out_d = nc.dram_tensor("out", [8, BF], F32, kind="ExternalOutput")

    with tile.TileContext(nc) as tc:
        with (
            tc.tile_pool(name="consts", bufs=1) as consts,
            tc.tile_pool(name="state", bufs=1) as state,
            tc.tile_pool(name="work", bufs=3) as work,
        ):
            lhsT = consts.tile([88, 320], F16)
            nc.sync.dma_start(out=lhsT[:], in_=lhsT_d.ap())
            biasS = consts.tile([80, 4], F32)
            nc.sync.dma_start(out=biasS[:], in_=bias_d.ap())
            maskS = consts.tile([80, 2], F32)
            nc.sync.dma_start(out=maskS[:], in_=mask_d.ap())
            vtab = consts.tile([40, L * G], F16)
            nc.sync.dma_start(out=vtab[:], in_=vtab_d.ap())

            # persistent state: rows 0:40 h1, 40:80 h0, 80:88 x
            hs = state.tile([88, BF], F16)
            cc = state.tile([80, BF], F16)   # cell state, rows [l1, l0]
            nc.vector.memset(hs[0:80, :], 0.0)
            nc.vector.memset(cc[:], 0.0)

            def xsrc(t):
                # x rows for step t: xT[c*L + t, hb*1024 + j]
                return bass.AP(tensor=xT.ap().tensor, offset=t * BC,
                               ap=[[L * BC, C], [BF, 2], [1, BF]])

            nc.sync.dma_start(out=hs[80:88, :], in_=xsrc(0))

            with tc.tile_pool(name="psum", bufs=1, space="PSUM") as gp:
                Pi = gp.tile([80, BF], F32, tag="Pi", name="Pi")
                Pf = gp.tile([80, BF], F32, tag="Pf", name="Pf")
                Pg = gp.tile([80, BF], F32, tag="Pg", name="Pg")
                Po = gp.tile([104, BF], F32, tag="Po", name="Po")
                acc = Po[96:104, :]

                for t in range(T):
                    if t in (W, W + 1):
                        # chunk-0 state reset via row mask (l0 at t=W,
                        # l1 at t=W+1)
                        mk = maskS[:, t - W:t - W + 1]
                        nc.vector.tensor_scalar(hs[0:80, :], hs[0:80, :],
                                                mk, None, MUL)
                        nc.vector.tensor_scalar(cc[:], cc[:], mk, None, MUL)

                    # js-minor order, f-gate first: the critical spine is
                    # f -> m2 -> cc -> tanh -> h, so get Pf finished ASAP.
                    gorder = [(Pf, 80), (Pi, 0), (Pg, 160), (Po, 240)]
                    for P, c0_ in gorder:
                        for js in range(J):
                            sl = slice(js * JS, (js + 1) * JS)
                            nc.tensor.matmul(P[0:80, sl],
                                             lhsT[:, c0_:c0_ + 80],
                                             hs[:, sl], start=True, stop=True)

                    # x prefetch for t+1 (overlaps this step's act/DVE)
                    if t + 1 < T:
                        nc.sync.dma_start(out=hs[80:88, :], in_=xsrc(t + 1))

                    Si = work.tile([80, BF], F16, tag="Si")
                    Sf = work.tile([80, BF], F16, tag="Sf")
                    Sg = work.tile([80, BF], F16, tag="Sg")
                    So = work.tile([80, BF], F16, tag="So")
                    m1 = work.tile([80, BF], F16, tag="m1")
                    m2 = work.tile([80, BF], F16, tag="m2")
                    Tt = work.tile([80, BF], F16, tag="Tt")
                    # Sf covers both halves in one act (early, off-chain);
                    # Si/Sg split per half so m1 js0 starts ~0.65us sooner.
                    nc.scalar.activation(Sf[:], Pf[0:80, :], Sig,
                                         bias=biasS[:, 1:2])
                    sl0 = slice(0, JS)
                    nc.scalar.activation(Si[:, sl0], Pi[0:80, sl0], Sig,
                                         bias=biasS[:, 0:1])
                    nc.scalar.activation(Sg[:, sl0], Pg[0:80, sl0], Tanh,
                                         bias=biasS[:, 2:3])

                    # c = f*c + i*g ; h = o*tanh(c)   (rows [l1, l0]);
                    # spine split into 512-wide halves so the two halves
                    # pipeline (next step's js0 matmuls only need h js0).
                    # The o-gate act is split too: So_js fills the Scalar
                    # gap while DVE computes cc_js, unblocking h earlier.
                    for js in range(J):
                        sl = slice(js * JS, (js + 1) * JS)
                        if js == 1:
                            # js1 halves of Si/Sg issued here so the js0
                            # spine's Scalar ops aren't queued behind them
                            nc.scalar.activation(Si[:, sl], Pi[0:80, sl],
                                                 Sig, bias=biasS[:, 0:1])
                            nc.scalar.activation(Sg[:, sl], Pg[0:80, sl],
                                                 Tanh, bias=biasS[:, 2:3])
                        nc.vector.tensor_tensor(m2[:, sl], Sf[:, sl],
                                                cc[:, sl], MUL)
                        nc.vector.tensor_tensor(m1[:, sl], Si[:, sl],
                                                Sg[:, sl], MUL)
                        nc.vector.tensor_tensor(cc[:, sl], m1[:, sl],
                                                m2[:, sl], ADD)
                        nc.scalar.activation(So[:, sl], Po[0:80, sl], Sig,
                                             bias=biasS[:, 3:4])
                        nc.scalar.activation(Tt[:, sl], cc[:, sl], Tanh)
                        nc.vector.tensor_tensor(hs[0:80, sl], So[:, sl],
                                                Tt[:, sl], MUL)

                    # readout accumulation (h1 of position t-(W+1))
                    tp = t - (W + 1)
                    if tp >= 0:
                        for js in range(J):
                            sl = slice(js * JS, (js + 1) * JS)
                            nc.tensor.matmul(
                                acc[:, sl], vtab[:, tp * G:(tp + 1) * G],
                                hs[0:40, sl],
                                start=(tp == 0), stop=(tp == L - 1),
                                tile_position=(0, 96))

                accs = work.tile([8, BF], F32, tag="accs")
                nc.scalar.copy(accs[:], acc[:])
                nc.sync.dma_start(out=out_d.ap(), in_=accs[:])

    nc.compile()
    _CACHE["nc"] = nc
    return nc


def _host_prep(inputs):
    """Build per-core input maps (host-side preprocessing)."""
    x = np.ascontiguousarray(inputs["x"].reshape(B, S).astype(np.float32))
    Wih0 = np.asarray(inputs["Wih0"], np.float32)
    Whh0 = np.asarray(inputs["Whh0"], np.float32)
    Wih1 = np.asarray(inputs["Wih1"], np.float32)
    Whh1 = np.asarray(inputs["Whh1"], np.float32)
    b0 = np.asarray(inputs["bih0"], np.float32) + np.asarray(inputs["bhh0"], np.float32)
    b1l = np.asarray(inputs["bih1"], np.float32) + np.asarray(inputs["bhh1"], np.float32)
    W1 = np.asarray(inputs["W1"], np.float32)
    b1 = np.asarray(inputs["b1"], np.float32)
    W2 = np.asarray(inputs["W2"], np.float32)
    b2 = np.asarray(inputs["b2"], np.float32)

    v2d = (W2 @ W1).reshape(S, H).astype(np.float32)
    c0 = float((W2 @ b1 + b2).reshape(-1)[0])

    # lhsT gate blocks (80 cols each): cols [l1: 5g+k | l0: 40+5g+k],
    # psum row = col index within block.  torch gate row = qi*H + k.
    lhsT = np.zeros((88, 320), np.float32)
    biasA = np.zeros((80, 4), np.float32)
    for qi in range(4):          # i, f, g, o
        col0 = qi * 80
        for g in range(G):
            for k in range(H):
                qrow = qi * H + k
                # layer 1 column (psum row 5g+k)
                c = col0 + 5 * g + k
                lhsT[5 * g:5 * g + 5, c] = Whh1[qrow, :]
                lhsT[40 + 5 * g:40 + 5 * g + 5, c] = Wih1[qrow, :]
                biasA[5 * g + k, qi] = b1l[qrow]
                # layer 0 column (psum row 40+5g+k)
                c = col0 + 40 + 5 * g + k
                lhsT[40 + 5 * g:40 + 5 * g + 5, c] = Whh0[qrow, :]
                lhsT[80 + g, c] = Wih0[qrow, 0]
                biasA[40 + 5 * g + k, qi] = b0[qrow]

    maskA = np.ones((80, 2), np.float32)
    maskA[40:50, 0] = 0.0        # t=W: zero chunk-0 layer-0 state
    maskA[0:10, 1] = 0.0         # t=W+1: zero chunk-0 layer-1 state

    vtab = np.zeros((40, L * G), np.float32)
    for tp in range(L):
        for g in range(G):
            s = (g // 2) * L + tp
            vtab[5 * g:5 * g + 5, tp * G + g] = v2d[s, :]

    lhsT = lhsT.astype(np.float16)
    vtab = vtab.astype(np.float16)

    in_maps = []
    for core in range(NCORES):
        xc = x[core * BC:(core + 1) * BC, :]          # (2048, 512)
        xTpad = np.zeros((W + S + 8, BC), np.float16)
        xTpad[W:W + S, :] = xc.T.astype(np.float16)
        in_maps.append({
            "xT": np.ascontiguousarray(xTpad),
            "lhsT": lhsT, "bias": biasA, "mask": maskA, "vtab": vtab,
        })
    return in_maps, c0


def _run(nc, in_maps):
    return run_bass_kernel_spmd(nc, in_maps, core_ids=list(range(NCORES)))


def _gather(res, c0):
    out = np.empty((B, 1), np.float32)
    for core in range(NCORES):
        a = np.asarray(res.results[core]["out"])      # (8, 1024)
        out[core * BC:core * BC + BF, 0] = a[0::2, :].sum(axis=0) + c0
        out[core * BC + BF:(core + 1) * BC, 0] = a[1::2, :].sum(axis=0) + c0
    return out


def kernel(**inputs):
    nc = _build_program()
    in_maps, c0 = _host_prep(inputs)
    res = _run(nc, in_maps)
    return _gather(res, c0)


# revision 18
# speedup vs baseline: 1.0150x; 1.0150x over previous
"""Trainium2 Bass kernel for nn_ConsumptionPredictor.

Model: 2-layer LSTM (H=5, S=512) over batch 16384, then two linear layers
with no intervening nonlinearity (W1/W2 collapse into a single 2560-dim
dot product v = W2 @ W1, c0 = W2 @ b1 + b2).

Strategy (per core, batch 2048 = 1/8 of 16384):
  * Sequence chunking: C=4 chunks of L=128 steps, each warmed up with W
    extra leading steps (forget-gate decay makes warmup error tiny).  All
    4 chunks x 2 batch halves advance in lockstep as G=8 groups of 1024
    samples -> virtual batch 8192 per step.
  * Both LSTM layers fused in one step loop (layer 1 lags layer 0 by one
    step).  Unified fp16 state tile hs (88, 1024): rows 0:40 h1 (8 groups
    x 5), 40:80 h0, 80:88 x (prefetched one step ahead by DMA).
  * Per-gate PSUM tiles (80, 1024) pack BOTH layers contiguously
    ([l1, l0]) so one matmul per gate per 512-slice computes both layers
    (lhsT packs Whh/Wih blocks, K=88), and every ACT/DVE op is a single
    base-0 80-row instruction (partition bases must be 0/32/64/96).
  * Scalar: 4 gate activations + tanh(c), each free-size 1024, with
    per-partition bias.  DVE: 4 fp16 tensor_tensor ops (2x mode).
  * Readout acc (out[b] += v[s].h1[b,s]) accumulated per step by a small
    matmul into rows 96:104 of the o-gate PSUM tile (fits the 8-bank
    budget); final 4-chunk reduction host-side.
  * Chunk-0 state resets at t=W / t=W+1 via per-partition mask multiplies
    (memset at partition base 40 would violate the alignment rule).
"""

import sys

import numpy as np

try:
    import concourse.bass as bass  # noqa: F401
except ImportError:  # pragma: no cover
    sys.path.insert(0, "/opt/trn_rl_repo")

import concourse.bass as bass
import concourse.tile as tile
from concourse import bacc, mybir
from concourse.bass_utils import run_bass_kernel_spmd

# ----- problem/config constants (hardcoded; kernel.py must be self-contained)
NCORES = 8
B, S, H = 16384, 512, 5
BC = B // NCORES          # 2048 samples per core
C = 4                     # sequence chunks
L = S // C                # 128 steps per chunk
W = 10                    # warmup steps per chunk
T = W + L + 1             # virtual steps (layer 1 lags by one)
G = 8                     # groups (4 chunks x 2 batch halves)
BF = 1024                 # free width per group
J = 2                     # 512-wide column slices (PSUM bank width)
JS = BF // J
F32 = mybir.dt.float32
F16 = mybir.dt.float16
Sig = mybir.ActivationFunctionType.Sigmoid
Tanh = mybir.ActivationFunctionType.Tanh
MUL = mybir.AluOpType.mult
ADD = mybir.AluOpType.add

_CACHE = {}


def _build_program():
    if "nc" in _CACHE:
        return _CACHE["nc"]

    nc = bacc.Bacc("TRN2", target_bir_lowering=False, debug=False,
                   enable_asserts=False, num_devices=NCORES)

    xT = nc.dram_tensor("xT", [W + S + 8, BC], F16, kind="ExternalInput")
    lhsT_d = nc.dram_tensor("lhsT", [88, 320], F16, kind="ExternalInput")
    bias_d = nc.dram_tensor("bias", [80, 4], F32, kind="ExternalInput")
    mask_d = nc.dram_tensor("mask", [80, 2], F32, kind="ExternalInput")
    vtab_d = nc.dram_tensor("vtab", [40, L * G], F16, kind="ExternalInput")
    out_d = nc.dram_tensor("out", [8, BF], F32, kind="ExternalOutput")

    with tile.TileContext(nc) as tc:
        with (
            tc.tile_pool(name="consts", bufs=1) as consts,
            tc.tile_pool(name="state", bufs=1) as state,
            tc.tile_pool(name="work", bufs=3) as work,
        ):
            lhsT = consts.tile([88, 320], F16)
            nc.sync.dma_start(out=lhsT[:], in_=lhsT_d.ap())
            biasS = consts.tile([80, 4], F32)
            nc.sync.dma_start(out=biasS[:], in_=bias_d.ap())
            maskS = consts.tile([80, 2], F32)
            nc.sync.dma_start(out=maskS[:], in_=mask_d.ap())
            vtab = consts.tile([40, L * G], F16)
            nc.sync.dma_start(out=vtab[:], in_=vtab_d.ap())

            # persistent state: rows 0:40 h1, 40:80 h0, 80:88 x
            hs = state.tile([88, BF], F16)
            cc = state.tile([80, BF], F16)   # cell state, rows [l1, l0]
            nc.vector.memset(hs[0:80, :], 0.0)
            nc.vector.memset(cc[:], 0.0)

            def xsrc(t):
                # x rows for step t: xT[c*L + t, hb*1024 + j]
                return bass.AP(tensor=xT.ap().tensor, offset=t * BC,
                               ap=[[L * BC, C], [BF, 2], [1, BF]])

            nc.sync.dma_start(out=hs[80:88, :], in_=xsrc(0))

            with tc.tile_pool(name="psum", bufs=1, space="PSUM") as gp:
                Pi = gp.tile([80, BF], F32, tag="Pi", name="Pi")
                Pf = gp.tile([80, BF], F32, tag="Pf", name="Pf")
                Pg = gp.tile([80, BF], F32, tag="Pg", name="Pg")
                Po = gp.tile([104, BF], F32, tag="Po", name="Po")
                acc = Po[96:104, :]

                for t in range(T):
                    if t in (W, W + 1):
                        # chunk-0 state reset via row mask (l0 at t=W,
                        # l1 at t=W+1)
                        mk = maskS[:, t - W:t - W + 1]
                        nc.vector.tensor_scalar(hs[0:80, :], hs[0:80, :],
                                                mk, None, MUL)
                        nc.vector.tensor_scalar(cc[:], cc[:], mk, None, MUL)

                    # js-minor order, f-gate first: the critical spine is
                    # f -> m2 -> cc -> tanh -> h, so get Pf finished ASAP.
                    gorder = [(Pf, 80), (Pi, 0), (Pg, 160), (Po, 240)]
                    for P, c0_ in gorder:
                        for js in range(J):
                            sl = slice(js * JS, (js + 1) * JS)
                            nc.tensor.matmul(P[0:80, sl],
                                             lhsT[:, c0_:c0_ + 80],
                                             hs[:, sl], start=True, stop=True)

                    # x prefetch for t+1 (overlaps this step's act/DVE)
                    if t + 1 < T:
                        nc.sync.dma_start(out=hs[80:88, :], in_=xsrc(t + 1))

                    Si = work.tile([80, BF], F16, tag="Si")
                    Sf = work.tile([80, BF], F16, tag="Sf")
                    Sg = work.tile([80, BF], F16, tag="Sg")
                    So = work.tile([80, BF], F16, tag="So")
                    m1 = work.tile([80, BF], F16, tag="m1")
                    m2 = work.tile([80, BF], F16, tag="m2")
                    Tt = work.tile([80, BF], F16, tag="Tt")
                    nc.scalar.activation(Sf[:], Pf[0:80, :], Sig,
                                         bias=biasS[:, 1:2])
                    nc.scalar.activation(Si[:], Pi[0:80, :], Sig,
                                         bias=biasS[:, 0:1])
                    nc.scalar.activation(Sg[:], Pg[0:80, :], Tanh,
                                         bias=biasS[:, 2:3])

                    # c = f*c + i*g ; h = o*tanh(c)   (rows [l1, l0]);
                    # spine split into 512-wide halves so the two halves
                    # pipeline (next step's js0 matmuls only need h js0).
                    # The o-gate act is split too: So_js fills the Scalar
                    # gap while DVE computes cc_js, unblocking h earlier.
                    for js in range(J):
                        sl = slice(js * JS, (js + 1) * JS)
                        nc.vector.tensor_tensor(m2[:, sl], Sf[:, sl],
                                                cc[:, sl], MUL)
                        nc.vector.tensor_tensor(m1[:, sl], Si[:, sl],
                                                Sg[:, sl], MUL)
                        nc.vector.tensor_tensor(cc[:, sl], m1[:, sl],
                                                m2[:, sl], ADD)
                        nc.scalar.activation(So[:, sl], Po[0:80, sl], Sig,
                                             bias=biasS[:, 3:4])
                        nc.scalar.activation(Tt[:, sl], cc[:, sl], Tanh)
                        nc.vector.tensor_tensor(hs[0:80, sl], So[:, sl],
                                                Tt[:, sl], MUL)

                    # readout accumulation (h1 of position t-(W+1))
                    tp = t - (W + 1)
                    if tp >= 0:
                        for js in range(J):
                            sl = slice(js * JS, (js + 1) * JS)
                            nc.tensor.matmul(
                                acc[:, sl], vtab[:, tp * G:(tp + 1) * G],
                                hs[0:40, sl],
                                start=(tp == 0), stop=(tp == L - 1),
                                tile_position=(0, 96))

                accs = work.tile([8, BF], F32, tag="accs")
                nc.scalar.copy(accs[:], acc[:])
                nc.sync.dma_start(out=out_d.ap(), in_=accs[:])

    nc.compile()
    _CACHE["nc"] = nc
    return nc


def _host_prep(inputs):
    """Build per-core input maps (host-side preprocessing)."""
    x = np.ascontiguousarray(inputs["x"].reshape(B, S).astype(np.float32))
    Wih0 = np.asarray(inputs["Wih0"], np.float32)
    Whh0 = np.asarray(inputs["Whh0"], np.float32)
    Wih1 = np.asarray(inputs["Wih1"], np.float32)
    Whh1 = np.asarray(inputs["Whh1"], np.float32)
    b0 = np.asarray(inputs["bih0"], np.float32) + np.asarray(inputs["bhh0"], np.float32)
    b1l = np.asarray(inputs["bih1"], np.float32) + np.asarray(inputs["bhh1"], np.float32)
    W1 = np.asarray(inputs["W1"], np.float32)
    b1 = np.asarray(inputs["b1"], np.float32)
    W2 = np.asarray(inputs["W2"], np.float32)
    b2 = np.asarray(inputs["b2"], np.float32)

    v2d = (W2 @ W1).reshape(S, H).astype(np.float32)
    c0 = float((W2 @ b1 + b2).reshape(-1)[0])

    # lhsT gate blocks (80 cols each): cols [l1: 5g+k | l0: 40+5g+k],
    # psum row = col index within block.  torch gate row = qi*H + k.
    lhsT = np.zeros((88, 320), np.float32)
    biasA = np.zeros((80, 4), np.float32)
    for qi in range(4):          # i, f, g, o
        col0 = qi * 80
        for g in range(G):
            for k in range(H):
                qrow = qi * H + k
                # layer 1 column (psum row 5g+k)
                c = col0 + 5 * g + k
                lhsT[5 * g:5 * g + 5, c] = Whh1[qrow, :]
                lhsT[40 + 5 * g:40 + 5 * g + 5, c] = Wih1[qrow, :]
                biasA[5 * g + k, qi] = b1l[qrow]
                # layer 0 column (psum row 40+5g+k)
                c = col0 + 40 + 5 * g + k
                lhsT[40 + 5 * g:40 + 5 * g + 5, c] = Whh0[qrow, :]
                lhsT[80 + g, c] = Wih0[qrow, 0]
                biasA[40 + 5 * g + k, qi] = b0[qrow]

    maskA = np.ones((80, 2), np.float32)
    maskA[40:50, 0] = 0.0        # t=W: zero chunk-0 layer-0 state
    maskA[0:10, 1] = 0.0         # t=W+1: zero chunk-0 layer-1 state

    vtab = np.zeros((40, L * G), np.float32)
    for tp in range(L):
        for g in range(G):
            s = (g // 2) * L + tp
            vtab[5 * g:5 * g + 5, tp * G + g] = v2d[s, :]

    lhsT = lhsT.astype(np.float16)
    vtab = vtab.astype(np.float16)

    in_maps = []
    for core in range(NCORES):
        xc = x[core * BC:(core + 1) * BC, :]          # (2048, 512)
        xTpad = np.zeros((W + S + 8, BC), np.float16)
        xTpad[W:W + S, :] = xc.T.astype(np.float16)
        in_maps.append({
            "xT": np.ascontiguousarray(xTpad),
            "lhsT": lhsT, "bias": biasA, "mask": maskA, "vtab": vtab,
        })
    return in_maps, c0


def _run(nc, in_maps):
    return run_bass_kernel_spmd(nc, in_maps, core_ids=list(range(NCORES)))


def _gather(res, c0):
    out = np.empty((B, 1), np.float32)
    for core in range(NCORES):
        a = np.asarray(res.results[core]["out"])      # (8, 1024)
        out[core * BC:core * BC + BF, 0] = a[0::2, :].sum(axis=0) + c0
        out[core * BC + BF:(core + 1) * BC, 0] = a[1::2, :].sum(axis=0) + c0
    return out


def kernel(**inputs):
    nc = _build_program()
    in_maps, c0 = _host_prep(inputs)
    res = _run(nc, in_maps)
    return _gather(res, c0)
